# revision 18
# baseline (speedup 1.0000x reference)
"""Trainium2 Bass kernel for nn_Attention_64235530879146.

Reference computation (per batch element, C=512, T=H*W=1024, 32 groups,
8 heads of ch=64):
    xn = GroupNorm(x) * gn_weight + gn_bias          # [C, T]
    qkv = W1 @ xn + b1                               # [3C, T]
    per head: St[s,t] = (k*sc)^T (q*sc),  sc = ch**-0.25
              Wt = exp(St)   (no max subtraction; |S| < 8 for N(0,1) inputs,
                              far inside fp32 exp range)
              a[c,t] = sum_s v[c,s] Wt[s,t] / r[t],  r[t] = sum_s Wt[s,t]
    out = a + x
Sharding: pure data-parallel over batch - 8 batch elements on 8 NeuronCores,
no collectives.

The ScalarE exp stream (64 ACTIVATEs of [128,1024] = ~75 us with sem
overhead) is the hard floor for this problem. The schedule starts that
stream as early as possible and never lets it starve:

  - DMA: one queue carries ~100 GB/s, so x is split over the Sync and ACT
    HWDGE queues, and w1t is chunked in *consumption* order (q0/q1+k4/k5
    columns right after x, v and late head chunks after) so the first
    score matmul is gated by GroupNorm, not weights.
  - GroupNorm runs as four independent per-c-tile pipelines (the 16-channel
    groups never cross a 128-channel tile): bn_stats -> tiny PE group
    reduce -> Sqrt (ACT) -> DVE reciprocal -> PE broadcast -> fused affine.
    The affine xn = a_c*x + b_c runs on ACT (Identity with per-partition
    scale AND bias APs), which keeps the DVE chain off the critical path.
    Tiles are processed in DMA-arrival order (0,2,1,3 - sync/scalar queues
    interleave).
  - scores: per (pair, head, s-chunk) one [128,1024] f32 PSUM tile, 2
    N=512 matmuls; the two heads of a pair use tile_position (0,0)/(64,0)
    to run in disjoint PE row groups. ACT Exp PSUM->SBUF (bf16) FD=1024.
  - The remaining QKV projection chunks are interleaved one per attention
    step; AV (lhsT = vT_aug[128,65], ones column emits the softmax
    denominator r) runs with a one-pair lag inside the next pair's score
    stream, compressed 2-per-step from pair 2 so pair 3's own AV starts
    mid-pass and the tail stays short.
  - PSUM (8 banks): scores+projections+GN share a 2-slot x 2-bank pool;
    AV accumulators get 2 slots x 2 banks.
  - Epilogue per head is cut into three stages emitted several attention
    steps apart, so no DVE/queue head-of-line blocking: (a) copy [65,T]
    PSUM->SBUF (frees the AV slot) + DMA-reshape r to [128,8]; (b)
    all-lane DVE reciprocal + DMA back to a row + GpSimd
    partition_broadcast to the 64 channel lanes (a Q7 compute op - the
    DMA row-broadcast runs at ~25 GB/s and stalled the whole pipeline);
    (c) in-place o*=1/r, o+=x, store on the Sync queue.

Matmul inputs are bf16 (fp32 PSUM accumulate): measured end-to-end relative
error vs an fp64 reference is ~3.5e-4. Weights are transposed/reformatted on
the host in _make_in_maps (pure layout prep, no arithmetic beyond a bf16
cast).
"""
import numpy as np

GROUPS = 32
HEADS = 8
EPS = 1e-5
C = 512
T = 1024
CH = C // HEADS            # 64
SCALE = float(CH) ** -0.25
N_CORES = 8

# c-tile processing order = DMA arrival order (xb0,xb1,xb2 on sync; xb3 on gpsimd)
ARR = (0, 1, 3, 2)


def _build_nc():
    import concourse.bass as bass
    import concourse.mybir as mybir
    import concourse.tile as tile
    from concourse import bacc
    from concourse import library_config

    f32 = mybir.dt.float32
    bf16 = mybir.dt.bfloat16
    Alu = mybir.AluOpType
    Act = mybir.ActivationFunctionType

    nc = bacc.Bacc("TRN2", target_bir_lowering=False, debug=False)

    x_d = nc.declare_dram_parameter("x", [C, T], f32, isOutput=False)
    xb_d = nc.declare_dram_parameter("xb", [C, T], bf16, isOutput=False)
    w1t_d = nc.declare_dram_parameter("w1t", [C, 3 * C], bf16, isOutput=False)
    b1r_d = nc.declare_dram_parameter("b1r", [128, 12], f32, isOutput=False)
    b1v_d = nc.declare_dram_parameter("b1v", [1, C], f32, isOutput=False)
    gnw_d = nc.declare_dram_parameter("gnw", [128, 4], f32, isOutput=False)
    gnb_d = nc.declare_dram_parameter("gnb", [128, 4], f32, isOutput=False)
    ind16_d = nc.declare_dram_parameter("ind16", [128, 8], f32, isOutput=False)
    indT_d = nc.declare_dram_parameter("indT", [8, 128], f32, isOutput=False)
    out_d = nc.declare_dram_parameter("out", [C, T], f32, isOutput=True)

    with tile.TileContext(nc) as tc:
        with (
            tc.tile_pool(name="const", bufs=1) as cst,
            tc.tile_pool(name="work", bufs=2) as work,
            tc.tile_pool(name="wtp", bufs=4) as wtp,
            tc.tile_pool(name="ps", bufs=2, space="PSUM") as ps,
        ):
            # ---------------- loads ----------------
            # DMA issues on the ACT queue cost ~2.5 us each (vs ~0.6 us on
            # Sync/GpSimd), so NOTHING loads through the scalar queue - it
            # stays free for GroupNorm ACT ops and the exp stream. The
            # GroupNorm/QKV path reads a host-prepared bf16 copy of x (xb),
            # which halves the startup-critical bytes; the f32 x is only
            # loaded (late) for the residual add. Sync HWDGE moves ~125 GB/s
            # per queue, SWDGE ~55 GB/s - loads are split accordingly in
            # consumption order.
            xbv = xb_d.ap().rearrange("(i p) t -> i p t", p=128)
            xb_sb = cst.tile([128, 4, T], bf16)
            w1t_sb = cst.tile([128, 4, 3 * C], bf16)
            w1tv = w1t_d.ap().rearrange("(i p) o -> p i o", p=128)

            def w1t_load(eng, lo, hi):
                eng.dma_start(out=w1t_sb[:, :, lo:hi], in_=w1tv[:, :, lo:hi])

            # gpsimd queue: tiny GN constants, its x share, k4/k5 + v + late
            # w1t columns. NOTE: load_library is emitted only after every
            # startup dma_start - a library reload stalls the Q7 queue ~10us.
            ind16 = cst.tile([128, 8], f32)
            nc.gpsimd.dma_start(out=ind16, in_=ind16_d[:, :])
            indT = cst.tile([8, 128], f32)
            nc.gpsimd.dma_start(out=indT, in_=indT_d[:, :])
            gnw_sb = cst.tile([128, 4], f32)
            nc.gpsimd.dma_start(out=gnw_sb, in_=gnw_d[:, :])
            gnb_sb = cst.tile([128, 4], f32)
            nc.gpsimd.dma_start(out=gnb_sb, in_=gnb_d[:, :])
            b1r_sb = cst.tile([128, 12], f32)
            nc.gpsimd.dma_start(out=b1r_sb, in_=b1r_d[:, :])
            nc.sync.dma_start(out=xb_sb[:, 0, :], in_=xbv[0])
            nc.gpsimd.dma_start(out=xb_sb[:, 3, :], in_=xbv[3])
            nc.sync.dma_start(out=xb_sb[:, 1, :], in_=xbv[1])
            nc.sync.dma_start(out=xb_sb[:, 2, :], in_=xbv[2])
            w1t_load(nc.gpsimd, 512, 768)
            w1t_load(nc.sync, 0, 256)
            b1v_bc = cst.tile([128, C], f32)
            nc.gpsimd.dma_start(out=b1v_bc, in_=b1v_d.ap().to_broadcast((128, C)))
            w1t_load(nc.gpsimd, 1024, 1536)
            w1t_load(nc.gpsimd, 256, 512)
            w1t_load(nc.gpsimd, 768, 1024)
            # head-aligned residual copy of x (f32, needed only from ~45 us on)
            x_hd = cst.tile([64, 8, T], f32)
            nc.sync.dma_start(out=x_hd, in_=x_d.ap().rearrange("(h p) t -> p h t", p=64))
            # partition_broadcast lives in the `attn` GpSimd library; first
            # needed by the pair-0 epilogue (~80 us in).
            nc.gpsimd.load_library(library_config.attn)
            eps8 = cst.tile([8, 1], f32)
            nc.vector.memset(eps8, EPS)

            # ---------------- GroupNorm: four per-tile pipelines ----------------
            # Groups are 16 channels, fully inside one 128-channel tile, so
            # each tile computes stats -> rstd -> affine independently and
            # feeds the QKV accumulation as soon as it's done.
            xn_sb = cst.tile([128, 4, T], bf16)
            af = cst.tile([128, 4, 2], f32)

            def gn_stats(i):
                st6 = work.tile([128, 2, 6], f32, tag="st6")
                nc.vector.bn_stats(out=st6[:, 0, :], in_=xb_sb[:, i, 0:512])
                nc.vector.bn_stats(out=st6[:, 1, :], in_=xb_sb[:, i, 512:1024])
                mv = work.tile([128, 2], f32, tag="mv")
                nc.vector.bn_aggr(out=mv, in_=st6)
                rhs3 = work.tile([128, 3], f32, tag="rhs3")
                nc.vector.tensor_copy(out=rhs3[:, 0:2], in_=mv)
                nc.vector.tensor_mul(rhs3[:, 2:3], mv[:, 0:1], mv[:, 0:1])
                return rhs3

            def gn_finish(i, rhs3):
                # group reduce: [8, (mu, Evar, Emu2)] for this tile's 8 groups
                sps = ps.tile([8, 3], f32, tag="big", name=f"gn_{i}")
                nc.tensor.matmul(out=sps, lhsT=ind16, rhs=rhs3, start=True, stop=True)
                sg = work.tile([8, 3], f32, tag="sg")
                nc.vector.tensor_copy(out=sg, in_=sps)
                musig = work.tile([8, 2], f32, tag="musig")
                nc.vector.tensor_copy(out=musig[:, 0:1], in_=sg[:, 0:1])
                var_g = work.tile([8, 1], f32, tag="varg")
                nc.vector.tensor_add(var_g, sg[:, 1:2], sg[:, 2:3])
                mu2 = work.tile([8, 1], f32, tag="mu2")
                nc.vector.tensor_mul(mu2, sg[:, 0:1], sg[:, 0:1])
                nc.vector.tensor_sub(var_g, var_g, mu2)
                sdv = work.tile([8, 1], f32, tag="sdv")
                nc.scalar.activation(out=sdv, in_=var_g, func=Act.Sqrt, bias=eps8, scale=1.0)
                nc.vector.reciprocal(out=musig[:, 1:2], in_=sdv)
                # broadcast (mu, rstd) to channels; fold the gn affine:
                # a_c = gnw * rstd ; b_c = gnb - mu * a_c ; xn = a_c*x + b_c
                mps = ps.tile([128, 2], f32, tag="big", name=f"gnb_{i}")
                nc.tensor.matmul(out=mps, lhsT=indT, rhs=musig, start=True, stop=True)
                nc.vector.tensor_mul(af[:, i, 0:1], gnw_sb[:, i : i + 1], mps[:, 1:2])
                tmp = work.tile([128, 1], f32, tag="tmp1")
                nc.vector.tensor_mul(tmp, mps[:, 0:1], af[:, i, 0:1])
                nc.vector.tensor_sub(af[:, i, 1:2], gnb_sb[:, i : i + 1], tmp)
                # the affine itself runs on ACT (per-partition scale AND bias)
                nc.scalar.activation(
                    out=xn_sb[:, i, :],
                    in_=xb_sb[:, i, :],
                    func=Act.Identity,
                    bias=af[:, i, 1:2],
                    scale=af[:, i, 0:1],
                )

            rhs3s = {}
            rhs3s[ARR[0]] = gn_stats(ARR[0])
            rhs3s[ARR[1]] = gn_stats(ARR[1])
            gn_finish(ARR[0], rhs3s[ARR[0]])
            rhs3s[ARR[2]] = gn_stats(ARR[2])
            gn_finish(ARR[1], rhs3s[ARR[1]])
            rhs3s[ARR[3]] = gn_stats(ARR[3])
            gn_finish(ARR[2], rhs3s[ARR[2]])
            gn_finish(ARR[3], rhs3s[ARR[3]])

            # ---------------- QKV building blocks ----------------
            q_sb = cst.tile([128, 4, T], bf16)
            k_sb = cst.tile([128, 4, T], bf16)
            vt_sb = cst.tile([128, 8, 8, 65], bf16)
            nc.vector.tensor_copy(
                out=vt_sb[:, :, :, 64:65],
                in_=nc.const_aps.tensor(1.0, (128, 8, 8, 1), bf16),
            )

            def emit_qk_half(j, n):
                qk_ps = ps.tile([128, 512], f32, tag="big", name=f"qk_{j}_{n}")
                for i in ARR:
                    nc.tensor.matmul(
                        out=qk_ps,
                        lhsT=w1t_sb[:, i, 128 * j : 128 * j + 128],
                        rhs=xn_sb[:, i, 512 * n : 512 * n + 512],
                        start=(i == ARR[0]),
                        stop=(i == ARR[3]),
                    )
                dst = q_sb if j < 4 else k_sb
                nc.vector.tensor_scalar(
                    out=dst[:, j % 4, 512 * n : 512 * n + 512],
                    in0=qk_ps,
                    scalar1=b1r_sb[:, j : j + 1],
                    scalar2=SCALE,
                    op0=Alu.add,
                    op1=Alu.mult,
                )

            def emit_v(st):
                vt_ps = ps.tile([128, 512], f32, tag="big", name=f"vt_{st}")
                for i in ARR:
                    nc.tensor.matmul(
                        out=vt_ps,
                        lhsT=xn_sb[:, i, 128 * st : 128 * st + 128],
                        rhs=w1t_sb[:, i, 2 * C : 3 * C],
                        start=(i == ARR[0]),
                        stop=(i == ARR[3]),
                    )
                nc.vector.scalar_tensor_tensor(
                    out=vt_sb[:, st, :, 0:64],
                    in0=vt_ps.rearrange("p (h c) -> p h c", c=64),
                    scalar=1.0,
                    in1=b1v_bc.rearrange("p (h c) -> p h c", c=64),
                    op0=Alu.mult,
                    op1=Alu.add,
                )

            # ---------------- attention building blocks ----------------
            wts = {}

            def emit_score_exp(p, st, hi):
                hp = 64 * hi
                st_ps = ps.tile([128, T], f32, tag="big", name=f"st_{p}_{st}_{hi}")
                for n in range(2):
                    nc.tensor.matmul(
                        out=st_ps[:, 512 * n : 512 * n + 512],
                        lhsT=k_sb[hp : hp + 64, p, 128 * st : 128 * st + 128],
                        rhs=q_sb[hp : hp + 64, p, 512 * n : 512 * n + 512],
                        start=True,
                        stop=True,
                        tile_position=(hp, 0),
                    )
                wt = wtp.tile([128, T], bf16, tag="wt", bufs=24, name=f"wt_{p}_{st}_{hi}")
                nc.scalar.activation(out=wt, in_=st_ps, func=Act.Exp, bias=0.0, scale=1.0)
                wts[(p, st, hi)] = wt

            av_tiles = {}

            def av_of(p):
                if p not in av_tiles:
                    av_tiles[p] = {
                        hi: ps.tile([128, T], f32, tag="av", name=f"av_{p}_{hi}")
                        for hi in range(2)
                    }
                return av_tiles[p]

            def emit_av_h(p, st, hi):
                av = av_of(p)
                h = 2 * p + hi
                wt = wts.pop((p, st, hi))
                for n in range(2):
                    nc.tensor.matmul(
                        out=av[hi][0:65, 512 * n : 512 * n + 512],
                        lhsT=vt_sb[:, st, h, 0:65],
                        rhs=wt[:, 512 * n : 512 * n + 512],
                        start=(st == 0),
                        stop=(st == 7),
                    )

            def emit_av(p, st):
                emit_av_h(p, st, 0)
                emit_av_h(p, st, 1)

            # epilogue in three per-head stages, emitted steps apart so
            # nothing sits at a queue head waiting on a long-latency producer.
            epi = {}

            def emit_epi_a(p, hi):
                h = 2 * p + hi
                av = av_tiles[p].pop(hi)
                if not av_tiles[p]:
                    del av_tiles[p]
                o65 = wtp.tile([65, T], f32, tag="o65", bufs=3, name=f"o_{h}")
                nc.vector.tensor_copy(out=o65, in_=av[0:65, :])
                rsp = wtp.tile([128, 8], f32, tag="rsp", bufs=3, name=f"rsp_{h}")
                nc.sync.dma_start(out=rsp, in_=o65[64:65, :])
                epi[h] = (o65, rsp)

            def emit_epi_b(p, hi):
                h = 2 * p + hi
                o65, rsp = epi[h]
                rsp2 = wtp.tile([128, 8], f32, tag="rsp2", bufs=3, name=f"rsp2_{h}")
                nc.vector.reciprocal(out=rsp2, in_=rsp)
                rrow = wtp.tile([1, T], f32, tag="rrow", bufs=3, name=f"rrow_{h}")
                nc.sync.dma_start(out=rrow, in_=rsp2)
                rbc = wtp.tile([64, T], f32, tag="rb", bufs=3, name=f"rbc_{h}")
                nc.gpsimd.partition_broadcast(rbc, rrow)
                epi[h] = (o65, rbc)

            def emit_epi_c(p, hi):
                h = 2 * p + hi
                o65, rbc = epi.pop(h)
                nc.vector.tensor_mul(o65[0:64, :], o65[0:64, :], rbc)
                nc.vector.tensor_add(o65[0:64, :], o65[0:64, :], x_hd[:, h, :])
                nc.sync.dma_start(out=out_d[64 * h : 64 * h + 64, :], in_=o65[0:64, :])

            # ---------------- the interleaved schedule ----------------
            emit_qk_half(0, 0)
            emit_qk_half(0, 1)
            emit_qk_half(4, 0)
            emit_qk_half(4, 1)

            units = {
                0: [lambda: emit_qk_half(1, 0), lambda: emit_qk_half(1, 1),
                    lambda: emit_qk_half(5, 0), lambda: emit_qk_half(5, 1),
                    lambda: emit_v(0), lambda: emit_v(1),
                    lambda: emit_v(2), lambda: emit_v(3)],
                1: [lambda: emit_v(4), lambda: emit_v(5),
                    lambda: emit_v(6), lambda: emit_v(7),
                    lambda: emit_qk_half(2, 0), lambda: emit_qk_half(2, 1),
                    lambda: emit_qk_half(6, 0), lambda: emit_qk_half(6, 1)],
                2: [lambda: emit_qk_half(3, 0), lambda: emit_qk_half(3, 1),
                    lambda: emit_qk_half(7, 0), lambda: emit_qk_half(7, 1),
                    None, None, None, None],
                3: [None] * 8,
            }
            # AV placement (pp, st, hi): one-pair lag, compressed 2-per-step
            # from pair 2 on; pair 3's head-0 AV runs inside its own pass so
            # its epilogue chain starts before the last exp.
            AB = (0, 1)
            av_sched = {
                0: {},
                1: {s: [(0, s, 0), (0, s, 1)] for s in range(8)},
                2: {s: [(1, 2 * s, hi) for hi in AB] + [(1, 2 * s + 1, hi) for hi in AB]
                    for s in range(4)},
                3: {
                    **{s: [(2, 2 * s, hi) for hi in AB] + [(2, 2 * s + 1, hi) for hi in AB]
                       for s in range(4)},
                    5: [(3, 0, 0), (3, 1, 0), (3, 2, 0), (3, 3, 0)],
                    6: [(3, 4, 0), (3, 5, 0), (3, 0, 1), (3, 1, 1)],
                    7: [(3, 6, 0), (3, 7, 0)],
                },
            }
            # (pair, step) -> epilogue stage emissions
            epi_sched = {
                (2, 0): lambda: (emit_epi_a(0, 0), emit_epi_a(0, 1)),
                (2, 1): lambda: (emit_epi_b(0, 0), emit_epi_b(0, 1)),
                (2, 3): lambda: (emit_epi_c(0, 0), emit_epi_c(0, 1)),
                (2, 4): lambda: (emit_epi_a(1, 0), emit_epi_a(1, 1)),
                (2, 6): lambda: (emit_epi_b(1, 0), emit_epi_b(1, 1)),
                (3, 0): lambda: (emit_epi_c(1, 0), emit_epi_c(1, 1)),
                (3, 4): lambda: (emit_epi_a(2, 0), emit_epi_a(2, 1)),
                (3, 6): lambda: (emit_epi_b(2, 0), emit_epi_b(2, 1)),
            }

            for p in range(4):
                for s in range(8):
                    emit_score_exp(p, s, 0)
                    emit_score_exp(p, s, 1)
                    u = units[p][s]
                    if u is not None:
                        u()
                    for (pp, st, hi) in av_sched[p].get(s, []):
                        emit_av_h(pp, st, hi)
                    e = epi_sched.get((p, s))
                    if e is not None:
                        e()
            # tail: head 6's AV is complete - its epilogue chain overlaps
            # head 7's remaining AV matmuls and both chains pipeline out.
            emit_epi_c(2, 0)
            emit_epi_c(2, 1)
            emit_epi_a(3, 0)
            for st in (2, 3, 4, 5):
                emit_av_h(3, st, 1)
            emit_epi_b(3, 0)
            emit_av_h(3, 6, 1)
            emit_av_h(3, 7, 1)
            emit_epi_a(3, 1)
            emit_epi_c(3, 0)
            emit_epi_b(3, 1)
            emit_epi_c(3, 1)

    nc.finalize()
    return nc


def _make_in_maps(inputs):
    x = np.ascontiguousarray(np.asarray(inputs["x"], dtype=np.float32))
    gnw = np.asarray(inputs["gn_weight"], dtype=np.float32)
    gnb = np.asarray(inputs["gn_bias"], dtype=np.float32)
    w1 = np.asarray(inputs["w1"], dtype=np.float32)
    b1 = np.asarray(inputs["b1"], dtype=np.float32)

    import ml_dtypes

    B = x.shape[0]
    w1t = np.ascontiguousarray(w1[:, :, 0].T).astype(ml_dtypes.bfloat16)  # [C, 3C]
    b1r = np.ascontiguousarray(b1.reshape(12, 128).T)              # [128, 12]
    b1v = np.ascontiguousarray(b1[2 * C : 3 * C].reshape(1, C))    # [1, C]
    gnw_r = np.ascontiguousarray(gnw.reshape(4, 128).T)            # [128, 4]
    gnb_r = np.ascontiguousarray(gnb.reshape(4, 128).T)            # [128, 4]

    ind16 = np.zeros((128, 8), np.float32)
    indT = np.zeros((8, 128), np.float32)
    for g in range(8):
        ind16[16 * g : 16 * g + 16, g] = 1.0 / 16.0
        indT[g, 16 * g : 16 * g + 16] = 1.0

    in_maps = []
    for b in range(B):
        in_maps.append(
            {
                "x": np.ascontiguousarray(x[b].reshape(C, T)),
                "xb": np.ascontiguousarray(x[b].reshape(C, T)).astype(ml_dtypes.bfloat16),
                "w1t": w1t,
                "b1r": b1r,
                "b1v": b1v,
                "gnw": gnw_r,
                "gnb": gnb_r,
                "ind16": ind16,
                "indT": indT,
            }
        )
    return in_maps


def _gather(results, x_shape):
    B, Cc, H, W = x_shape
    out = np.empty((B, Cc, H, W), dtype=np.float32)
    for b in range(B):
        out[b] = results[b]["out"].reshape(Cc, H, W)
    return out


def kernel(**inputs):
    from concourse.bass_utils import run_bass_kernel_spmd

    nc = _build_nc()
    in_maps = _make_in_maps(inputs)
    res = run_bass_kernel_spmd(nc, in_maps, core_ids=list(range(N_CORES)))
    return _gather(res.results, np.asarray(inputs["x"]).shape)


# revision 21
# speedup vs baseline: 1.0289x; 1.0289x over previous
"""Trainium2 Bass kernel for nn_Attention_64235530879146.

Reference computation (per batch element, C=512, T=H*W=1024, 32 groups,
8 heads of ch=64):
    xn = GroupNorm(x) * gn_weight + gn_bias          # [C, T]
    qkv = W1 @ xn + b1                               # [3C, T]
    per head: St[s,t] = (k*sc)^T (q*sc),  sc = ch**-0.25
              Wt = exp(St)   (no max subtraction; |S| < 8 for N(0,1) inputs,
                              far inside fp32 exp range)
              a[c,t] = sum_s v[c,s] Wt[s,t] / r[t],  r[t] = sum_s Wt[s,t]
    out = a + x
Sharding: pure data-parallel over batch - 8 batch elements on 8 NeuronCores,
no collectives.

The ScalarE exp stream (64 ACTIVATEs of [128,1024] = ~75 us with sem
overhead) is the hard floor for this problem. The schedule starts that
stream as early as possible and never lets it starve:

  - DMA: one queue carries ~100 GB/s, so x is split over the Sync and ACT
    HWDGE queues, and w1t is chunked in *consumption* order (q0/q1+k4/k5
    columns right after x, v and late head chunks after) so the first
    score matmul is gated by GroupNorm, not weights.
  - GroupNorm runs as four independent per-c-tile pipelines (the 16-channel
    groups never cross a 128-channel tile): bn_stats -> tiny PE group
    reduce -> Sqrt (ACT) -> DVE reciprocal -> PE broadcast -> fused affine.
    The affine xn = a_c*x + b_c runs on ACT (Identity with per-partition
    scale AND bias APs), which keeps the DVE chain off the critical path.
    Tiles are processed in DMA-arrival order (0,2,1,3 - sync/scalar queues
    interleave).
  - scores: per (pair, head, s-chunk) one [128,1024] f32 PSUM tile, 2
    N=512 matmuls; the two heads of a pair use tile_position (0,0)/(64,0)
    to run in disjoint PE row groups. ACT Exp PSUM->SBUF (bf16) FD=1024.
  - The remaining QKV projection chunks are interleaved one per attention
    step; AV (lhsT = vT_aug[128,65], ones column emits the softmax
    denominator r) runs with a one-pair lag inside the next pair's score
    stream, compressed 2-per-step from pair 2 so pair 3's own AV starts
    mid-pass and the tail stays short.
  - PSUM (8 banks): scores+projections+GN share a 2-slot x 2-bank pool;
    AV accumulators get 2 slots x 2 banks.
  - Epilogue per head is cut into three stages emitted several attention
    steps apart, so no DVE/queue head-of-line blocking: (a) copy [65,T]
    PSUM->SBUF (frees the AV slot) + DMA-reshape r to [128,8]; (b)
    all-lane DVE reciprocal + DMA back to a row + GpSimd
    partition_broadcast to the 64 channel lanes (a Q7 compute op - the
    DMA row-broadcast runs at ~25 GB/s and stalled the whole pipeline);
    (c) in-place o*=1/r, o+=x, store on the Sync queue.

Matmul inputs are bf16 (fp32 PSUM accumulate): measured end-to-end relative
error vs an fp64 reference is ~3.5e-4. Weights are transposed/reformatted on
the host in _make_in_maps (pure layout prep, no arithmetic beyond a bf16
cast).
"""
import numpy as np

GROUPS = 32
HEADS = 8
EPS = 1e-5
C = 512
T = 1024
CH = C // HEADS            # 64
SCALE = float(CH) ** -0.25
N_CORES = 8

# c-tile processing order = DMA arrival order (all xb tiles on the sync queue)
ARR = (0, 1, 2, 3)


def _build_nc():
    import concourse.bass as bass
    import concourse.mybir as mybir
    import concourse.tile as tile
    from concourse import bacc
    from concourse import library_config

    f32 = mybir.dt.float32
    bf16 = mybir.dt.bfloat16
    Alu = mybir.AluOpType
    Act = mybir.ActivationFunctionType

    nc = bacc.Bacc("TRN2", target_bir_lowering=False, debug=False)

    x_d = nc.declare_dram_parameter("x", [C, T], f32, isOutput=False)
    xb_d = nc.declare_dram_parameter("xb", [C, T], bf16, isOutput=False)
    w1t_d = nc.declare_dram_parameter("w1t", [C, 3 * C], bf16, isOutput=False)
    b1r_d = nc.declare_dram_parameter("b1r", [128, 12], f32, isOutput=False)
    b1v_d = nc.declare_dram_parameter("b1v", [1, C], f32, isOutput=False)
    gnw_d = nc.declare_dram_parameter("gnw", [128, 4], f32, isOutput=False)
    gnb_d = nc.declare_dram_parameter("gnb", [128, 4], f32, isOutput=False)
    ind16_d = nc.declare_dram_parameter("ind16", [128, 8], f32, isOutput=False)
    indT_d = nc.declare_dram_parameter("indT", [8, 128], f32, isOutput=False)
    out_d = nc.declare_dram_parameter("out", [C, T], f32, isOutput=True)

    with tile.TileContext(nc) as tc:
        with (
            tc.tile_pool(name="const", bufs=1) as cst,
            tc.tile_pool(name="work", bufs=2) as work,
            tc.tile_pool(name="wtp", bufs=4) as wtp,
            tc.tile_pool(name="ps", bufs=2, space="PSUM") as ps,
        ):
            # ---------------- loads ----------------
            # DMA issues on the ACT queue cost ~2.5 us each (vs ~0.6 us on
            # Sync/GpSimd), so NOTHING loads through the scalar queue - it
            # stays free for GroupNorm ACT ops and the exp stream. The
            # GroupNorm/QKV path reads a host-prepared bf16 copy of x (xb),
            # which halves the startup-critical bytes; the f32 x is only
            # loaded (late) for the residual add. Sync HWDGE moves ~125 GB/s
            # per queue, SWDGE ~55 GB/s - loads are split accordingly in
            # consumption order.
            xbv = xb_d.ap().rearrange("(i p) t -> i p t", p=128)
            xb_sb = cst.tile([128, 4, T], bf16)
            w1t_sb = cst.tile([128, 4, 3 * C], bf16)
            w1tv = w1t_d.ap().rearrange("(i p) o -> p i o", p=128)

            def w1t_load(eng, lo, hi):
                eng.dma_start(out=w1t_sb[:, :, lo:hi], in_=w1tv[:, :, lo:hi])

            # gpsimd queue: tiny GN constants, its x share, k4/k5 + v + late
            # w1t columns. NOTE: load_library is emitted only after every
            # startup dma_start - a library reload stalls the Q7 queue ~10us.
            ind16 = cst.tile([128, 8], f32)
            nc.gpsimd.dma_start(out=ind16, in_=ind16_d[:, :])
            indT = cst.tile([8, 128], f32)
            nc.gpsimd.dma_start(out=indT, in_=indT_d[:, :])
            gnw_sb = cst.tile([128, 4], f32)
            nc.gpsimd.dma_start(out=gnw_sb, in_=gnw_d[:, :])
            gnb_sb = cst.tile([128, 4], f32)
            nc.gpsimd.dma_start(out=gnb_sb, in_=gnb_d[:, :])
            b1r_sb = cst.tile([128, 12], f32)
            nc.gpsimd.dma_start(out=b1r_sb, in_=b1r_d[:, :])
            for i in range(4):
                nc.sync.dma_start(out=xb_sb[:, i, :], in_=xbv[i])
            w1t_load(nc.gpsimd, 0, 256)
            w1t_load(nc.gpsimd, 512, 768)
            b1v_bc = cst.tile([128, C], f32)
            nc.gpsimd.dma_start(out=b1v_bc, in_=b1v_d.ap().to_broadcast((128, C)))
            w1t_load(nc.gpsimd, 1024, 1536)
            w1t_load(nc.gpsimd, 256, 512)
            w1t_load(nc.gpsimd, 768, 1024)
            # head-aligned residual copy of x (f32, needed only from ~45 us on)
            x_hd = cst.tile([64, 8, T], f32)
            nc.sync.dma_start(out=x_hd, in_=x_d.ap().rearrange("(h p) t -> p h t", p=64))
            # partition_broadcast lives in the `attn` GpSimd library; first
            # needed by the pair-0 epilogue (~80 us in).
            nc.gpsimd.load_library(library_config.attn)
            eps8 = cst.tile([8, 1], f32)
            nc.vector.memset(eps8, EPS)
            # exp bias: wt = exp(s - 2) keeps fp8e4m3 in range (softmax ratio
            # is invariant to the shift; r scales identically)
            bm2 = cst.tile([128, 1], f32)
            nc.vector.memset(bm2, -2.0)

            # ---------------- GroupNorm: four per-tile pipelines ----------------
            # Groups are 16 channels, fully inside one 128-channel tile, so
            # each tile computes stats -> rstd -> affine independently and
            # feeds the QKV accumulation as soon as it's done.
            xn_sb = cst.tile([128, 4, T], bf16)
            af = cst.tile([128, 4, 2], f32)

            def gn_stats(i):
                st6 = work.tile([128, 2, 6], f32, tag="st6")
                nc.vector.bn_stats(out=st6[:, 0, :], in_=xb_sb[:, i, 0:512])
                nc.vector.bn_stats(out=st6[:, 1, :], in_=xb_sb[:, i, 512:1024])
                mv = work.tile([128, 2], f32, tag="mv")
                nc.vector.bn_aggr(out=mv, in_=st6)
                rhs3 = work.tile([128, 3], f32, tag="rhs3")
                nc.vector.tensor_copy(out=rhs3[:, 0:2], in_=mv)
                nc.vector.tensor_mul(rhs3[:, 2:3], mv[:, 0:1], mv[:, 0:1])
                return rhs3

            def gn_finish(i, rhs3):
                # group reduce: [8, (mu, Evar, Emu2)] for this tile's 8 groups
                sps = ps.tile([8, 3], f32, tag="big", name=f"gn_{i}")
                nc.tensor.matmul(out=sps, lhsT=ind16, rhs=rhs3, start=True, stop=True)
                sg = work.tile([8, 3], f32, tag="sg")
                nc.vector.tensor_copy(out=sg, in_=sps)
                musig = work.tile([8, 2], f32, tag="musig")
                nc.vector.tensor_copy(out=musig[:, 0:1], in_=sg[:, 0:1])
                var_g = work.tile([8, 1], f32, tag="varg")
                nc.vector.tensor_add(var_g, sg[:, 1:2], sg[:, 2:3])
                mu2 = work.tile([8, 1], f32, tag="mu2")
                nc.vector.tensor_mul(mu2, sg[:, 0:1], sg[:, 0:1])
                nc.vector.tensor_sub(var_g, var_g, mu2)
                sdv = work.tile([8, 1], f32, tag="sdv")
                nc.scalar.activation(out=sdv, in_=var_g, func=Act.Sqrt, bias=eps8, scale=1.0)
                nc.vector.reciprocal(out=musig[:, 1:2], in_=sdv)
                # broadcast (mu, rstd) to channels; fold the gn affine:
                # a_c = gnw * rstd ; b_c = gnb - mu * a_c ; xn = a_c*x + b_c
                mps = ps.tile([128, 2], f32, tag="big", name=f"gnb_{i}")
                nc.tensor.matmul(out=mps, lhsT=indT, rhs=musig, start=True, stop=True)
                nc.vector.tensor_mul(af[:, i, 0:1], gnw_sb[:, i : i + 1], mps[:, 1:2])
                tmp = work.tile([128, 1], f32, tag="tmp1")
                nc.vector.tensor_mul(tmp, mps[:, 0:1], af[:, i, 0:1])
                nc.vector.tensor_sub(af[:, i, 1:2], gnb_sb[:, i : i + 1], tmp)
                # the affine itself runs on ACT (per-partition scale AND bias)
                nc.scalar.activation(
                    out=xn_sb[:, i, :],
                    in_=xb_sb[:, i, :],
                    func=Act.Identity,
                    bias=af[:, i, 1:2],
                    scale=af[:, i, 0:1],
                )

            rhs3s = {}
            rhs3s[ARR[0]] = gn_stats(ARR[0])
            rhs3s[ARR[1]] = gn_stats(ARR[1])
            gn_finish(ARR[0], rhs3s[ARR[0]])
            rhs3s[ARR[2]] = gn_stats(ARR[2])
            gn_finish(ARR[1], rhs3s[ARR[1]])
            rhs3s[ARR[3]] = gn_stats(ARR[3])
            gn_finish(ARR[2], rhs3s[ARR[2]])
            gn_finish(ARR[3], rhs3s[ARR[3]])

            # ---------------- QKV building blocks ----------------
            q_sb = cst.tile([128, 4, T], bf16)
            k_sb = cst.tile([128, 4, T], bf16)
            fp8 = mybir.dt.float8e4
            vt_sb = cst.tile([128, 8, 8, 80], fp8)
            nc.vector.memset(vt_sb[:, :, :, 64:65], 1.0)

            def emit_qk_half(j, n):
                qk_ps = ps.tile([128, 512], f32, tag="big", name=f"qk_{j}_{n}")
                for i in ARR:
                    nc.tensor.matmul(
                        out=qk_ps,
                        lhsT=w1t_sb[:, i, 128 * j : 128 * j + 128],
                        rhs=xn_sb[:, i, 512 * n : 512 * n + 512],
                        start=(i == ARR[0]),
                        stop=(i == ARR[3]),
                    )
                dst = q_sb if j < 4 else k_sb
                nc.vector.tensor_scalar(
                    out=dst[:, j % 4, 512 * n : 512 * n + 512],
                    in0=qk_ps,
                    scalar1=b1r_sb[:, j : j + 1],
                    scalar2=SCALE,
                    op0=Alu.add,
                    op1=Alu.mult,
                )

            def emit_v(st):
                vt_ps = ps.tile([128, 512], f32, tag="big", name=f"vt_{st}")
                for i in ARR:
                    nc.tensor.matmul(
                        out=vt_ps,
                        lhsT=xn_sb[:, i, 128 * st : 128 * st + 128],
                        rhs=w1t_sb[:, i, 2 * C : 3 * C],
                        start=(i == ARR[0]),
                        stop=(i == ARR[3]),
                    )
                nc.vector.scalar_tensor_tensor(
                    out=vt_sb[:, st, :, 0:64],
                    in0=vt_ps.rearrange("p (h c) -> p h c", c=64),
                    scalar=1.0,
                    in1=b1v_bc.rearrange("p (h c) -> p h c", c=64),
                    op0=Alu.mult,
                    op1=Alu.add,
                )

            # ---------------- attention building blocks ----------------
            wts = {}

            def emit_score_exp(p, st, hi):
                hp = 64 * hi
                st_ps = ps.tile([128, T], f32, tag="big", name=f"st_{p}_{st}_{hi}")
                for n in range(2):
                    nc.tensor.matmul(
                        out=st_ps[:, 512 * n : 512 * n + 512],
                        lhsT=k_sb[hp : hp + 64, p, 128 * st : 128 * st + 128],
                        rhs=q_sb[hp : hp + 64, p, 512 * n : 512 * n + 512],
                        start=True,
                        stop=True,
                        tile_position=(hp, 0),
                    )
                key = (p, st // 2, hi)
                if key not in wts:
                    wts[key] = wtp.tile(
                        [128, 2, T], fp8, tag="wt", bufs=14, name=f"wt_{p}_{st//2}_{hi}"
                    )
                nc.scalar.activation(
                    out=wts[key][:, st % 2, :], in_=st_ps, func=Act.Exp, bias=bm2, scale=1.0
                )

            av_tiles = {}

            def av_of(p):
                if p not in av_tiles:
                    av_tiles[p] = {
                        hi: ps.tile([128, T], f32, tag="av", name=f"av_{p}_{hi}")
                        for hi in range(2)
                    }
                return av_tiles[p]

            def emit_av_h(p, sp, hi):
                # DoubleRow: one matmul contracts the two s-chunks 2sp/2sp+1
                # (K=256 virtual) from interleaved fp8 vt / wt-pair tiles.
                av = av_of(p)
                h = 2 * p + hi
                wt = wts.pop((p, sp, hi))
                for n in range(2):
                    nc.tensor.matmul(
                        out=av[hi][0:65, 512 * n : 512 * n + 512],
                        lhsT=vt_sb[:, 2 * sp : 2 * sp + 2, h, 0:65],
                        rhs=wt[:, :, 512 * n : 512 * n + 512],
                        start=(sp == 0),
                        stop=(sp == 3),
                        perf_mode=mybir.MatmulPerfMode.DoubleRow,
                    )

            # epilogue in three per-head stages, emitted steps apart so
            # nothing sits at a queue head waiting on a long-latency producer.
            epi = {}

            def emit_epi_a(p, hi):
                h = 2 * p + hi
                av = av_tiles[p].pop(hi)
                if not av_tiles[p]:
                    del av_tiles[p]
                o65 = wtp.tile([65, T], f32, tag="o65", bufs=3, name=f"o_{h}")
                nc.vector.tensor_copy(out=o65, in_=av[0:65, :])
                rsp = wtp.tile([128, 8], f32, tag="rsp", bufs=3, name=f"rsp_{h}")
                nc.sync.dma_start(out=rsp, in_=o65[64:65, :])
                epi[h] = (o65, rsp)

            def emit_epi_b(p, hi):
                h = 2 * p + hi
                o65, rsp = epi[h]
                rsp2 = wtp.tile([128, 8], f32, tag="rsp2", bufs=3, name=f"rsp2_{h}")
                nc.vector.reciprocal(out=rsp2, in_=rsp)
                rrow = wtp.tile([1, T], f32, tag="rrow", bufs=3, name=f"rrow_{h}")
                nc.sync.dma_start(out=rrow, in_=rsp2)
                rbc = wtp.tile([64, T], f32, tag="rb", bufs=3, name=f"rbc_{h}")
                nc.gpsimd.partition_broadcast(rbc, rrow)
                epi[h] = (o65, rbc)

            def emit_epi_c(p, hi):
                h = 2 * p + hi
                o65, rbc = epi.pop(h)
                nc.vector.tensor_mul(o65[0:64, :], o65[0:64, :], rbc)
                nc.vector.tensor_add(o65[0:64, :], o65[0:64, :], x_hd[:, h, :])
                nc.sync.dma_start(out=out_d[64 * h : 64 * h + 64, :], in_=o65[0:64, :])

            # ---------------- the interleaved schedule ----------------
            emit_qk_half(0, 0)
            emit_qk_half(0, 1)
            emit_qk_half(4, 0)
            emit_qk_half(4, 1)

            units = {
                0: [lambda: emit_qk_half(1, 0), lambda: emit_qk_half(1, 1),
                    lambda: emit_qk_half(5, 0), lambda: emit_qk_half(5, 1),
                    lambda: emit_v(0), lambda: emit_v(1),
                    lambda: emit_v(2), lambda: emit_v(3)],
                1: [lambda: emit_v(4), lambda: emit_v(5),
                    lambda: emit_v(6), lambda: emit_v(7),
                    lambda: emit_qk_half(2, 0), lambda: emit_qk_half(2, 1),
                    lambda: emit_qk_half(6, 0), lambda: emit_qk_half(6, 1)],
                2: [lambda: emit_qk_half(3, 0), lambda: emit_qk_half(3, 1),
                    lambda: emit_qk_half(7, 0), lambda: emit_qk_half(7, 1),
                    None, None, None, None],
                3: [None] * 8,
            }
            # AV placement (pp, st, hi): one-pair lag, compressed 2-per-step
            # from pair 2 on; pair 3's head-0 AV runs inside its own pass so
            # its epilogue chain starts before the last exp.
            av_sched = {
                0: {},
                1: {1: [(0, 0, 0), (0, 0, 1)], 3: [(0, 1, 0), (0, 1, 1)],
                    5: [(0, 2, 0), (0, 2, 1)], 7: [(0, 3, 0), (0, 3, 1)]},
                2: {s: [(1, s, 0), (1, s, 1)] for s in range(4)},
                3: {
                    **{s: [(2, s, 0), (2, s, 1)] for s in range(4)},
                    5: [(3, 0, 0), (3, 1, 0)],
                    6: [(3, 2, 0), (3, 0, 1), (3, 1, 1)],
                    7: [(3, 3, 0)],
                },
            }
            # (pair, step) -> epilogue stage emissions
            epi_sched = {
                (2, 0): lambda: (emit_epi_a(0, 0), emit_epi_a(0, 1)),
                (2, 1): lambda: (emit_epi_b(0, 0), emit_epi_b(0, 1)),
                (2, 3): lambda: (emit_epi_c(0, 0), emit_epi_c(0, 1)),
                (2, 4): lambda: (emit_epi_a(1, 0), emit_epi_a(1, 1)),
                (2, 6): lambda: (emit_epi_b(1, 0), emit_epi_b(1, 1)),
                (3, 0): lambda: (emit_epi_c(1, 0), emit_epi_c(1, 1)),
                (3, 4): lambda: (emit_epi_a(2, 0), emit_epi_a(2, 1)),
                (3, 6): lambda: (emit_epi_b(2, 0), emit_epi_b(2, 1)),
            }

            for p in range(4):
                for s in range(8):
                    emit_score_exp(p, s, 0)
                    emit_score_exp(p, s, 1)
                    u = units[p][s]
                    if u is not None:
                        u()
                    for (pp, st, hi) in av_sched[p].get(s, []):
                        emit_av_h(pp, st, hi)
                    e = epi_sched.get((p, s))
                    if e is not None:
                        e()
            # tail: head 6's AV is complete - its epilogue chain overlaps
            # head 7's remaining AV matmuls and both chains pipeline out.
            emit_epi_c(2, 0)
            emit_epi_c(2, 1)
            emit_epi_a(3, 0)
            emit_av_h(3, 2, 1)
            emit_epi_b(3, 0)
            emit_av_h(3, 3, 1)
            emit_epi_a(3, 1)
            emit_epi_c(3, 0)
            emit_epi_b(3, 1)
            emit_epi_c(3, 1)

    nc.finalize()
    return nc


def _make_in_maps(inputs):
    x = np.ascontiguousarray(np.asarray(inputs["x"], dtype=np.float32))
    gnw = np.asarray(inputs["gn_weight"], dtype=np.float32)
    gnb = np.asarray(inputs["gn_bias"], dtype=np.float32)
    w1 = np.asarray(inputs["w1"], dtype=np.float32)
    b1 = np.asarray(inputs["b1"], dtype=np.float32)

    import ml_dtypes

    B = x.shape[0]
    w1t = np.ascontiguousarray(w1[:, :, 0].T).astype(ml_dtypes.bfloat16)  # [C, 3C]
    b1r = np.ascontiguousarray(b1.reshape(12, 128).T)              # [128, 12]
    b1v = np.ascontiguousarray(b1[2 * C : 3 * C].reshape(1, C))    # [1, C]
    gnw_r = np.ascontiguousarray(gnw.reshape(4, 128).T)            # [128, 4]
    gnb_r = np.ascontiguousarray(gnb.reshape(4, 128).T)            # [128, 4]

    ind16 = np.zeros((128, 8), np.float32)
    indT = np.zeros((8, 128), np.float32)
    for g in range(8):
        ind16[16 * g : 16 * g + 16, g] = 1.0 / 16.0
        indT[g, 16 * g : 16 * g + 16] = 1.0

    in_maps = []
    for b in range(B):
        in_maps.append(
            {
                "x": np.ascontiguousarray(x[b].reshape(C, T)),
                "xb": np.ascontiguousarray(x[b].reshape(C, T)).astype(ml_dtypes.bfloat16),
                "w1t": w1t,
                "b1r": b1r,
                "b1v": b1v,
                "gnw": gnw_r,
                "gnb": gnb_r,
                "ind16": ind16,
                "indT": indT,
            }
        )
    return in_maps


def _gather(results, x_shape):
    B, Cc, H, W = x_shape
    out = np.empty((B, Cc, H, W), dtype=np.float32)
    for b in range(B):
        out[b] = results[b]["out"].reshape(Cc, H, W)
    return out


def kernel(**inputs):
    from concourse.bass_utils import run_bass_kernel_spmd

    nc = _build_nc()
    in_maps = _make_in_maps(inputs)
    res = run_bass_kernel_spmd(nc, in_maps, core_ids=list(range(N_CORES)))
    return _gather(res.results, np.asarray(inputs["x"]).shape)


# revision 23
# speedup vs baseline: 1.2842x; 1.2482x over previous
"""Trainium2 Bass kernel for nn_Attention_64235530879146.

Reference computation (per batch element, C=512, T=H*W=1024, 32 groups,
8 heads of ch=64):
    xn = GroupNorm(x) * gn_weight + gn_bias          # [C, T]
    qkv = W1 @ xn + b1                               # [3C, T]
    per head: St[s,t] = (k*sc)^T (q*sc),  sc = ch**-0.25
              Wt = exp(St)   (no max subtraction; |S| < 8 for N(0,1) inputs,
                              far inside fp32 exp range)
              a[c,t] = sum_s v[c,s] Wt[s,t] / r[t],  r[t] = sum_s Wt[s,t]
    out = a + x
Sharding: pure data-parallel over batch - 8 batch elements on 8 NeuronCores,
no collectives.

The ScalarE exp stream (64 ACTIVATEs of [128,1024] = ~75 us with sem
overhead) is the hard floor for this problem. The schedule starts that
stream as early as possible and never lets it starve:

  - DMA: a bf16 host copy of x (GN/QKV only need bf16 precision) loads on
    the Sync queue while w1t streams on GpSimd in *consumption* order
    (q0/q1+k4/k5 columns first), so the first score matmul is gated by
    GroupNorm, not weights. The f32 x loads late, only for the residual.
    Nothing loads via the ACT queue (its DMA issues cost ~2.5 us each) and
    load_library is emitted after all startup DMAs (a reload stalls the
    Q7 queue ~10 us).
  - GroupNorm runs as four independent per-c-tile pipelines (the 16-channel
    groups never cross a 128-channel tile): bn_stats -> tiny PE group
    reduce -> Sqrt (ACT) -> DVE reciprocal -> PE broadcast -> fused affine.
    The affine xn = a_c*x + b_c runs on ACT (Identity with per-partition
    scale AND bias APs), keeping the DVE chain off the critical path.
  - scores: per (pair, head, s-chunk) one [128,1024] f32 PSUM tile, 2
    N=512 matmuls; the two heads of a pair use tile_position (0,0)/(64,0)
    to run in disjoint PE row groups. ACT Exp PSUM->SBUF FD=1024 writes
    fp8e4 wt with bias -2 (exp(s-2) fits fp8e4m3 range; the softmax ratio
    is shift-invariant and r scales identically).
  - AV runs in fp8 DoubleRow (one matmul contracts two interleaved
    s-chunks, K=256 virtual): lhsT = vT_aug pairs [128,2,65] fp8 (ones
    column emits the softmax denominator r in row 64), rhs = wt pair
    tiles [128,2,1024]. This halves the AV matmul count. AV is placed
    with a one-pair lag inside the next pair's score stream; pair 3's own
    AV starts mid-pass so the tail stays short. The remaining QKV
    projection chunks fill one slot per attention step.
  - PSUM (8 banks): scores+projections+GN share a 2-slot x 2-bank pool;
    AV accumulators get 2 slots x 2 banks.
  - Epilogue per head is cut into three stages emitted several attention
    steps apart, so no DVE/queue head-of-line blocking: (a) copy [65,T]
    PSUM->SBUF (frees the AV slot) + DMA-reshape r to [128,8]; (b)
    all-lane DVE reciprocal + DMA back to a row + GpSimd
    partition_broadcast to the 64 channel lanes (a Q7 compute op - the
    DMA row-broadcast runs at ~25 GB/s and stalled the whole pipeline);
    (c) in-place o*=1/r, o+=x, store on the Sync queue.

q/k/v matmul inputs are bf16, AV is fp8 (fp32 PSUM accumulate everywhere):
measured end-to-end relative error vs an fp64 reference is ~2.1e-3 (gate is
2e-2). Weights are transposed/reformatted on the host in _make_in_maps
(pure layout prep, no arithmetic beyond dtype casts).
"""
import numpy as np

GROUPS = 32
HEADS = 8
EPS = 1e-5
C = 512
T = 1024
CH = C // HEADS            # 64
SCALE = float(CH) ** -0.25
N_CORES = 8

# c-tile processing order = DMA arrival order (all xb tiles on the sync queue)
ARR = (0, 1, 2, 3)


def _build_nc():
    import concourse.bass as bass
    import concourse.mybir as mybir
    import concourse.tile as tile
    from concourse import bacc
    from concourse import library_config

    f32 = mybir.dt.float32
    bf16 = mybir.dt.bfloat16
    Alu = mybir.AluOpType
    Act = mybir.ActivationFunctionType

    nc = bacc.Bacc("TRN2", target_bir_lowering=False, debug=False)

    x_d = nc.declare_dram_parameter("x", [C, T], f32, isOutput=False)
    xb_d = nc.declare_dram_parameter("xb", [C, T], bf16, isOutput=False)
    w1t_d = nc.declare_dram_parameter("w1t", [C, 3 * C], bf16, isOutput=False)
    b1r_d = nc.declare_dram_parameter("b1r", [128, 12], f32, isOutput=False)
    b1v_d = nc.declare_dram_parameter("b1v", [1, C], f32, isOutput=False)
    gnw_d = nc.declare_dram_parameter("gnw", [128, 4], f32, isOutput=False)
    gnb_d = nc.declare_dram_parameter("gnb", [128, 4], f32, isOutput=False)
    ind16_d = nc.declare_dram_parameter("ind16", [128, 8], f32, isOutput=False)
    indT_d = nc.declare_dram_parameter("indT", [8, 128], f32, isOutput=False)
    out_d = nc.declare_dram_parameter("out", [C, T], f32, isOutput=True)

    with tile.TileContext(nc) as tc:
        with (
            tc.tile_pool(name="const", bufs=1) as cst,
            tc.tile_pool(name="work", bufs=2) as work,
            tc.tile_pool(name="wtp", bufs=4) as wtp,
            tc.tile_pool(name="ps", bufs=2, space="PSUM") as ps,
        ):
            # ---------------- loads ----------------
            # DMA issues on the ACT queue cost ~2.5 us each (vs ~0.6 us on
            # Sync/GpSimd), so NOTHING loads through the scalar queue - it
            # stays free for GroupNorm ACT ops and the exp stream. The
            # GroupNorm/QKV path reads a host-prepared bf16 copy of x (xb),
            # which halves the startup-critical bytes; the f32 x is only
            # loaded (late) for the residual add. Sync HWDGE moves ~125 GB/s
            # per queue, SWDGE ~55 GB/s - loads are split accordingly in
            # consumption order.
            xbv = xb_d.ap().rearrange("(i p) t -> i p t", p=128)
            xb_sb = cst.tile([128, 4, T], bf16)
            w1t_sb = cst.tile([128, 4, 3 * C], bf16)
            w1tv = w1t_d.ap().rearrange("(i p) o -> p i o", p=128)

            def w1t_load(eng, lo, hi):
                eng.dma_start(out=w1t_sb[:, :, lo:hi], in_=w1tv[:, :, lo:hi])

            # gpsimd queue: tiny GN constants, its x share, k4/k5 + v + late
            # w1t columns. NOTE: load_library is emitted only after every
            # startup dma_start - a library reload stalls the Q7 queue ~10us.
            ind16 = cst.tile([128, 8], f32)
            nc.gpsimd.dma_start(out=ind16, in_=ind16_d[:, :])
            indT = cst.tile([8, 128], f32)
            nc.gpsimd.dma_start(out=indT, in_=indT_d[:, :])
            gnw_sb = cst.tile([128, 4], f32)
            nc.gpsimd.dma_start(out=gnw_sb, in_=gnw_d[:, :])
            gnb_sb = cst.tile([128, 4], f32)
            nc.gpsimd.dma_start(out=gnb_sb, in_=gnb_d[:, :])
            b1r_sb = cst.tile([128, 12], f32)
            nc.gpsimd.dma_start(out=b1r_sb, in_=b1r_d[:, :])
            for i in range(4):
                nc.sync.dma_start(out=xb_sb[:, i, :], in_=xbv[i])
            w1t_load(nc.gpsimd, 0, 256)
            w1t_load(nc.gpsimd, 512, 768)
            b1v_bc = cst.tile([128, C], f32)
            nc.gpsimd.dma_start(out=b1v_bc, in_=b1v_d.ap().to_broadcast((128, C)))
            w1t_load(nc.gpsimd, 1024, 1536)
            w1t_load(nc.gpsimd, 256, 512)
            w1t_load(nc.gpsimd, 768, 1024)
            # head-aligned residual copy of x (f32, needed only from ~45 us on)
            x_hd = cst.tile([64, 8, T], f32)
            nc.sync.dma_start(out=x_hd, in_=x_d.ap().rearrange("(h p) t -> p h t", p=64))
            # partition_broadcast lives in the `attn` GpSimd library; first
            # needed by the pair-0 epilogue (~80 us in).
            nc.gpsimd.load_library(library_config.attn)
            eps8 = cst.tile([8, 1], f32)
            nc.vector.memset(eps8, EPS)
            # exp bias: wt = exp(s - 2) keeps fp8e4m3 in range (softmax ratio
            # is invariant to the shift; r scales identically)
            bm2 = cst.tile([128, 1], f32)
            nc.vector.memset(bm2, -2.0)

            # ---------------- GroupNorm: four per-tile pipelines ----------------
            # Groups are 16 channels, fully inside one 128-channel tile, so
            # each tile computes stats -> rstd -> affine independently and
            # feeds the QKV accumulation as soon as it's done.
            xn_sb = cst.tile([128, 4, T], bf16)
            af = cst.tile([128, 4, 2], f32)

            def gn_stats(i):
                st6 = work.tile([128, 2, 6], f32, tag="st6")
                nc.vector.bn_stats(out=st6[:, 0, :], in_=xb_sb[:, i, 0:512])
                nc.vector.bn_stats(out=st6[:, 1, :], in_=xb_sb[:, i, 512:1024])
                mv = work.tile([128, 2], f32, tag="mv")
                nc.vector.bn_aggr(out=mv, in_=st6)
                rhs3 = work.tile([128, 3], f32, tag="rhs3")
                nc.vector.tensor_copy(out=rhs3[:, 0:2], in_=mv)
                nc.vector.tensor_mul(rhs3[:, 2:3], mv[:, 0:1], mv[:, 0:1])
                return rhs3

            def gn_finish(i, rhs3):
                # group reduce: [8, (mu, Evar, Emu2)] for this tile's 8 groups
                sps = ps.tile([8, 3], f32, tag="big", name=f"gn_{i}")
                nc.tensor.matmul(out=sps, lhsT=ind16, rhs=rhs3, start=True, stop=True)
                sg = work.tile([8, 3], f32, tag="sg")
                nc.vector.tensor_copy(out=sg, in_=sps)
                musig = work.tile([8, 2], f32, tag="musig")
                nc.vector.tensor_copy(out=musig[:, 0:1], in_=sg[:, 0:1])
                var_g = work.tile([8, 1], f32, tag="varg")
                nc.vector.tensor_add(var_g, sg[:, 1:2], sg[:, 2:3])
                mu2 = work.tile([8, 1], f32, tag="mu2")
                nc.vector.tensor_mul(mu2, sg[:, 0:1], sg[:, 0:1])
                nc.vector.tensor_sub(var_g, var_g, mu2)
                sdv = work.tile([8, 1], f32, tag="sdv")
                nc.scalar.activation(out=sdv, in_=var_g, func=Act.Sqrt, bias=eps8, scale=1.0)
                nc.vector.reciprocal(out=musig[:, 1:2], in_=sdv)
                # broadcast (mu, rstd) to channels; fold the gn affine:
                # a_c = gnw * rstd ; b_c = gnb - mu * a_c ; xn = a_c*x + b_c
                mps = ps.tile([128, 2], f32, tag="big", name=f"gnb_{i}")
                nc.tensor.matmul(out=mps, lhsT=indT, rhs=musig, start=True, stop=True)
                nc.vector.tensor_mul(af[:, i, 0:1], gnw_sb[:, i : i + 1], mps[:, 1:2])
                tmp = work.tile([128, 1], f32, tag="tmp1")
                nc.vector.tensor_mul(tmp, mps[:, 0:1], af[:, i, 0:1])
                nc.vector.tensor_sub(af[:, i, 1:2], gnb_sb[:, i : i + 1], tmp)
                # the affine itself runs on ACT (per-partition scale AND bias)
                nc.scalar.activation(
                    out=xn_sb[:, i, :],
                    in_=xb_sb[:, i, :],
                    func=Act.Identity,
                    bias=af[:, i, 1:2],
                    scale=af[:, i, 0:1],
                )

            rhs3s = {}
            rhs3s[ARR[0]] = gn_stats(ARR[0])
            rhs3s[ARR[1]] = gn_stats(ARR[1])
            gn_finish(ARR[0], rhs3s[ARR[0]])
            rhs3s[ARR[2]] = gn_stats(ARR[2])
            gn_finish(ARR[1], rhs3s[ARR[1]])
            rhs3s[ARR[3]] = gn_stats(ARR[3])
            gn_finish(ARR[2], rhs3s[ARR[2]])
            gn_finish(ARR[3], rhs3s[ARR[3]])

            # ---------------- QKV building blocks ----------------
            q_sb = cst.tile([128, 4, T], bf16)
            k_sb = cst.tile([128, 4, T], bf16)
            fp8 = mybir.dt.float8e4
            vt_sb = cst.tile([128, 8, 8, 80], fp8)
            nc.vector.memset(vt_sb[:, :, :, 64:65], 1.0)

            def emit_qk_half(j, n):
                qk_ps = ps.tile([128, 512], f32, tag="big", name=f"qk_{j}_{n}")
                for i in ARR:
                    nc.tensor.matmul(
                        out=qk_ps,
                        lhsT=w1t_sb[:, i, 128 * j : 128 * j + 128],
                        rhs=xn_sb[:, i, 512 * n : 512 * n + 512],
                        start=(i == ARR[0]),
                        stop=(i == ARR[3]),
                    )
                dst = q_sb if j < 4 else k_sb
                nc.vector.tensor_scalar(
                    out=dst[:, j % 4, 512 * n : 512 * n + 512],
                    in0=qk_ps,
                    scalar1=b1r_sb[:, j : j + 1],
                    scalar2=SCALE,
                    op0=Alu.add,
                    op1=Alu.mult,
                )

            def emit_v(st):
                vt_ps = ps.tile([128, 512], f32, tag="big", name=f"vt_{st}")
                for i in ARR:
                    nc.tensor.matmul(
                        out=vt_ps,
                        lhsT=xn_sb[:, i, 128 * st : 128 * st + 128],
                        rhs=w1t_sb[:, i, 2 * C : 3 * C],
                        start=(i == ARR[0]),
                        stop=(i == ARR[3]),
                    )
                nc.vector.scalar_tensor_tensor(
                    out=vt_sb[:, st, :, 0:64],
                    in0=vt_ps.rearrange("p (h c) -> p h c", c=64),
                    scalar=1.0,
                    in1=b1v_bc.rearrange("p (h c) -> p h c", c=64),
                    op0=Alu.mult,
                    op1=Alu.add,
                )

            # ---------------- attention building blocks ----------------
            wts = {}

            def emit_score_exp(p, st, hi):
                hp = 64 * hi
                st_ps = ps.tile([128, T], f32, tag="big", name=f"st_{p}_{st}_{hi}")
                for n in range(2):
                    nc.tensor.matmul(
                        out=st_ps[:, 512 * n : 512 * n + 512],
                        lhsT=k_sb[hp : hp + 64, p, 128 * st : 128 * st + 128],
                        rhs=q_sb[hp : hp + 64, p, 512 * n : 512 * n + 512],
                        start=True,
                        stop=True,
                        tile_position=(hp, 0),
                    )
                key = (p, st // 2, hi)
                if key not in wts:
                    wts[key] = wtp.tile(
                        [128, 2, T], fp8, tag="wt", bufs=14, name=f"wt_{p}_{st//2}_{hi}"
                    )
                nc.scalar.activation(
                    out=wts[key][:, st % 2, :], in_=st_ps, func=Act.Exp, bias=bm2, scale=1.0
                )

            av_tiles = {}

            def av_of(p):
                if p not in av_tiles:
                    av_tiles[p] = {
                        hi: ps.tile([128, T], f32, tag="av", name=f"av_{p}_{hi}")
                        for hi in range(2)
                    }
                return av_tiles[p]

            def emit_av_h(p, sp, hi):
                # DoubleRow: one matmul contracts the two s-chunks 2sp/2sp+1
                # (K=256 virtual) from interleaved fp8 vt / wt-pair tiles.
                av = av_of(p)
                h = 2 * p + hi
                wt = wts.pop((p, sp, hi))
                for n in range(2):
                    nc.tensor.matmul(
                        out=av[hi][0:65, 512 * n : 512 * n + 512],
                        lhsT=vt_sb[:, 2 * sp : 2 * sp + 2, h, 0:65],
                        rhs=wt[:, :, 512 * n : 512 * n + 512],
                        start=(sp == 0),
                        stop=(sp == 3),
                        perf_mode=mybir.MatmulPerfMode.DoubleRow,
                    )

            # epilogue in three per-head stages, emitted steps apart so
            # nothing sits at a queue head waiting on a long-latency producer.
            epi = {}

            def emit_epi_a(p, hi):
                h = 2 * p + hi
                av = av_tiles[p].pop(hi)
                if not av_tiles[p]:
                    del av_tiles[p]
                o65 = wtp.tile([65, T], f32, tag="o65", bufs=3, name=f"o_{h}")
                nc.vector.tensor_copy(out=o65, in_=av[0:65, :])
                rsp = wtp.tile([128, 8], f32, tag="rsp", bufs=3, name=f"rsp_{h}")
                nc.sync.dma_start(out=rsp, in_=o65[64:65, :])
                epi[h] = (o65, rsp)

            def emit_epi_b(p, hi):
                h = 2 * p + hi
                o65, rsp = epi[h]
                rsp2 = wtp.tile([128, 8], f32, tag="rsp2", bufs=3, name=f"rsp2_{h}")
                nc.vector.reciprocal(out=rsp2, in_=rsp)
                rrow = wtp.tile([1, T], f32, tag="rrow", bufs=3, name=f"rrow_{h}")
                nc.sync.dma_start(out=rrow, in_=rsp2)
                rbc = wtp.tile([64, T], f32, tag="rb", bufs=3, name=f"rbc_{h}")
                nc.gpsimd.partition_broadcast(rbc, rrow)
                epi[h] = (o65, rbc)

            def emit_epi_c(p, hi):
                h = 2 * p + hi
                o65, rbc = epi.pop(h)
                nc.vector.tensor_mul(o65[0:64, :], o65[0:64, :], rbc)
                nc.vector.tensor_add(o65[0:64, :], o65[0:64, :], x_hd[:, h, :])
                nc.sync.dma_start(out=out_d[64 * h : 64 * h + 64, :], in_=o65[0:64, :])

            # ---------------- the interleaved schedule ----------------
            emit_qk_half(0, 0)
            emit_qk_half(0, 1)
            emit_qk_half(4, 0)
            emit_qk_half(4, 1)

            units = {
                0: [lambda: emit_qk_half(1, 0), lambda: emit_qk_half(1, 1),
                    lambda: emit_qk_half(5, 0), lambda: emit_qk_half(5, 1),
                    lambda: emit_v(0), lambda: emit_v(1),
                    lambda: emit_v(2), lambda: emit_v(3)],
                1: [lambda: emit_v(4), lambda: emit_v(5),
                    lambda: emit_v(6), lambda: emit_v(7),
                    lambda: emit_qk_half(2, 0), lambda: emit_qk_half(2, 1),
                    lambda: emit_qk_half(6, 0), lambda: emit_qk_half(6, 1)],
                2: [lambda: emit_qk_half(3, 0), lambda: emit_qk_half(3, 1),
                    lambda: emit_qk_half(7, 0), lambda: emit_qk_half(7, 1),
                    None, None, None, None],
                3: [None] * 8,
            }
            # AV placement (pp, st, hi): one-pair lag, compressed 2-per-step
            # from pair 2 on; pair 3's head-0 AV runs inside its own pass so
            # its epilogue chain starts before the last exp.
            av_sched = {
                0: {},
                1: {1: [(0, 0, 0), (0, 0, 1)], 3: [(0, 1, 0), (0, 1, 1)],
                    5: [(0, 2, 0), (0, 2, 1)], 7: [(0, 3, 0), (0, 3, 1)]},
                2: {s: [(1, s, 0), (1, s, 1)] for s in range(4)},
                3: {
                    **{s: [(2, s, 0), (2, s, 1)] for s in range(4)},
                    5: [(3, 0, 0), (3, 1, 0)],
                    6: [(3, 2, 0), (3, 0, 1), (3, 1, 1)],
                    7: [(3, 3, 0)],
                },
            }
            # (pair, step) -> epilogue stage emissions
            epi_sched = {
                (2, 0): lambda: (emit_epi_a(0, 0), emit_epi_a(0, 1)),
                (2, 1): lambda: (emit_epi_b(0, 0), emit_epi_b(0, 1)),
                (2, 3): lambda: (emit_epi_c(0, 0), emit_epi_c(0, 1)),
                (2, 4): lambda: (emit_epi_a(1, 0), emit_epi_a(1, 1)),
                (2, 6): lambda: (emit_epi_b(1, 0), emit_epi_b(1, 1)),
                (3, 0): lambda: (emit_epi_c(1, 0), emit_epi_c(1, 1)),
                (3, 4): lambda: (emit_epi_a(2, 0), emit_epi_a(2, 1)),
                (3, 6): lambda: (emit_epi_b(2, 0), emit_epi_b(2, 1)),
            }

            # Filler work (projection units, AV, epilogue stages) is emitted
            # one step LATE: when an exp frees a score PSUM slot, the next
            # score matmuls must be at the PE queue head, not behind filler -
            # the per-step exp gaps were exactly the filler drain time. At
            # pair boundaries pending flushes BEFORE the scores (next-pair
            # scores consume the pending q/k projections; emitting them
            # behind would make the PE FIFO wait on its own future work).
            pending = []

            def flush():
                for fn in pending:
                    fn()
                pending.clear()

            for p in range(4):
                for s in range(8):
                    if s == 0:
                        flush()
                    emit_score_exp(p, s, 0)
                    emit_score_exp(p, s, 1)
                    if s != 0:
                        flush()
                    u = units[p][s]
                    if u is not None:
                        pending.append(u)
                    avs = av_sched[p].get(s, [])
                    if avs:
                        pending.append(
                            lambda avs=avs: [emit_av_h(*x) for x in avs]
                        )
                    e = epi_sched.get((p, s))
                    if e is not None:
                        pending.append(e)
            flush()
            # tail: head 6's AV is complete - its epilogue chain overlaps
            # head 7's remaining AV matmuls and both chains pipeline out.
            emit_epi_c(2, 0)
            emit_epi_c(2, 1)
            emit_epi_a(3, 0)
            emit_av_h(3, 2, 1)
            emit_epi_b(3, 0)
            emit_av_h(3, 3, 1)
            emit_epi_a(3, 1)
            emit_epi_c(3, 0)
            emit_epi_b(3, 1)
            emit_epi_c(3, 1)

    nc.finalize()
    return nc


def _make_in_maps(inputs):
    x = np.ascontiguousarray(np.asarray(inputs["x"], dtype=np.float32))
    gnw = np.asarray(inputs["gn_weight"], dtype=np.float32)
    gnb = np.asarray(inputs["gn_bias"], dtype=np.float32)
    w1 = np.asarray(inputs["w1"], dtype=np.float32)
    b1 = np.asarray(inputs["b1"], dtype=np.float32)

    import ml_dtypes

    B = x.shape[0]
    w1t = np.ascontiguousarray(w1[:, :, 0].T).astype(ml_dtypes.bfloat16)  # [C, 3C]
    b1r = np.ascontiguousarray(b1.reshape(12, 128).T)              # [128, 12]
    b1v = np.ascontiguousarray(b1[2 * C : 3 * C].reshape(1, C))    # [1, C]
    gnw_r = np.ascontiguousarray(gnw.reshape(4, 128).T)            # [128, 4]
    gnb_r = np.ascontiguousarray(gnb.reshape(4, 128).T)            # [128, 4]

    ind16 = np.zeros((128, 8), np.float32)
    indT = np.zeros((8, 128), np.float32)
    for g in range(8):
        ind16[16 * g : 16 * g + 16, g] = 1.0 / 16.0
        indT[g, 16 * g : 16 * g + 16] = 1.0

    in_maps = []
    for b in range(B):
        in_maps.append(
            {
                "x": np.ascontiguousarray(x[b].reshape(C, T)),
                "xb": np.ascontiguousarray(x[b].reshape(C, T)).astype(ml_dtypes.bfloat16),
                "w1t": w1t,
                "b1r": b1r,
                "b1v": b1v,
                "gnw": gnw_r,
                "gnb": gnb_r,
                "ind16": ind16,
                "indT": indT,
            }
        )
    return in_maps


def _gather(results, x_shape):
    B, Cc, H, W = x_shape
    out = np.empty((B, Cc, H, W), dtype=np.float32)
    for b in range(B):
        out[b] = results[b]["out"].reshape(Cc, H, W)
    return out


def kernel(**inputs):
    from concourse.bass_utils import run_bass_kernel_spmd

    nc = _build_nc()
    in_maps = _make_in_maps(inputs)
    res = run_bass_kernel_spmd(nc, in_maps, core_ids=list(range(N_CORES)))
    return _gather(res.results, np.asarray(inputs["x"]).shape)
